# revision 1
# baseline (speedup 1.0000x reference)
"""Trainium2 Bass kernel: frequency-domain regularized (Wiener) deconvolution.

Reference computation (B=16, T=8192, C=8, FIL=16):
    h  = fft(w_real + i*w_imag)            # (FIL, T)
    g  = conj(h) / (|h|^2 + s)             # (FIL, T)
    xf = fft(x, axis=T)                    # per (b, c) row
    y  = real(ifft(xf[:,None,:,:] * g[None,:,None,:]))
    out = y -> (B, T, FIL*C) + bias

Sharding: data-parallel over batch across 8 cores (2 batches/core); filter
params replicated.  FFTs are 4-step Cooley-Tukey matmuls on the PE array
(T = N2*N1, N2=128, N1=64; n = n1 + N1*n2, k = k2 + N2*k1):

  forward:  M1 (contract n2, fp32r) -> twiddle W^(n1 k2) (DVE, broadcast AP)
            -> PE transpose T1 -> M2 (contract n1, stacked-complex K)
            -> Z0 [k1r;k1i | (row,k2)]
  filter:   G = conj(H)/(|H|^2+s) computed on-device from w/s via the same
            forward machinery; assembled into stacked tiles [Gr;Gr], [-Gi;Gi]
  inverse:  3-op complex multiply by G (stacked-swap trick) -> M3 (contract
            k1) -> downcast bf16 -> PE transpose T2 -> M4 per-n1' with the
            inverse twiddle folded into static bf16 weights; bias added on
            PSUM evacuation; direct strided DMA to the output layout.
"""
import sys

sys.path.insert(0, "/opt/trn_rl_repo")

import numpy as np


def _get_cc():
    import concourse.bacc as bacc
    import concourse.mybir as mybir
    import concourse.tile as tile
    return bacc, mybir, tile


class Cfg:
    def __init__(self, T=8192, N2=128, N1=64, BL=2, C=8, FIL=16):
        assert N1 * N2 == T
        self.T, self.N2, self.N1, self.BL, self.C, self.FIL = T, N2, N1, BL, C, FIL
        self.ROWS = BL * C
        self.FC = FIL * C


FULL = Cfg()


def host_consts(cfg):
    """Static (input-independent) weights, as fp32 numpy arrays."""
    T, N1, N2 = cfg.T, cfg.N1, cfg.N2
    f32 = np.float32
    cs = {}
    a2 = np.arange(N2)
    a1 = np.arange(N1)
    F2 = np.exp(-2j * np.pi * np.outer(a2, a2) / N2)        # [n2, k2]
    cs["c_F2r"] = F2.real.astype(f32)
    cs["c_F2i"] = F2.imag.astype(f32)
    cs["c_F2in"] = (-F2.imag).astype(f32)
    Tw = np.exp(-2j * np.pi * np.outer(a2, a1) / T)         # [k2, n1]
    cs["c_Twr"] = Tw.real.astype(f32)
    cs["c_Twi"] = Tw.imag.astype(f32)
    cs["c_Twin"] = (-Tw.imag).astype(f32)
    F1 = np.exp(-2j * np.pi * np.outer(a1, a1) / N1)        # [n1, k1]
    cs["c_M2"] = np.hstack([np.vstack([F1.real, -F1.imag]),
                            np.vstack([F1.imag, F1.real])]).astype(f32)
    Fb1 = np.exp(2j * np.pi * np.outer(a1, a1) / N1)        # [k1, n1']
    cs["c_M3"] = np.hstack([np.vstack([Fb1.real, -Fb1.imag]),
                            np.vstack([Fb1.imag, Fb1.real])]).astype(f32)
    # M4 per-n1' weights, inverse twiddle folded in:
    #   L_{n1'}[k2, n2'] = exp(+2j pi k2 n2'/N2) * exp(+2j pi n1' k2 / T) / T
    Fb2 = np.exp(2j * np.pi * np.outer(a2, a2) / N2)        # [k2, n2']
    ph = np.exp(2j * np.pi * np.outer(a1, a2) / T)          # [n1', k2]
    L = Fb2[None, :, :] * ph[:, :, None] / T                # [n1', k2, n2']
    Lr = L.real.transpose(1, 0, 2).reshape(N2, N1 * N2)     # [k2, (n1', n2')]
    Lin = (-L.imag).transpose(1, 0, 2).reshape(N2, N1 * N2)
    cs["c_L"] = np.concatenate([Lr, Lin], axis=1).astype(f32)  # [k2 | (ri, n1', n2')]
    cs["c_idr"] = np.eye(N2, dtype=f32)
    cs["c_ones"] = np.ones((1, 1), dtype=f32)  # resized at input time
    cs["c_idb"] = np.eye(2 * N1, dtype=f32)
    return cs


def build_nc(cfg, debug_dumps=False):
    bacc, mybir, tile = _get_cc()
    F32, F32R, BF16 = mybir.dt.float32, mybir.dt.float32r, mybir.dt.bfloat16
    AL = mybir.AluOpType
    T, N1, N2, BL, C, FIL = cfg.T, cfg.N1, cfg.N2, cfg.BL, cfg.C, cfg.FIL
    ROWS, FC = cfg.ROWS, cfg.FC
    N1s = 2 * N1          # stacked (real; imag) partition dim
    KF = FIL * N2         # H/G free size, (f, k2) order
    RN = ROWS * N2        # Z0 free size, (row, k2) order
    KB = C * N2           # per-(b,f) inverse free size, (c, k2) order
    MCH = 512             # matmul free-dim chunk (one PSUM bank of fp32)

    nc = bacc.Bacc("TRN2", debug=False)

    xs_d = nc.dram_tensor("xs", [BL, T, C], F32R, kind="ExternalInput")
    wr_d = nc.dram_tensor("wr", [FIL, T], F32R, kind="ExternalInput")
    wi_d = nc.dram_tensor("wi", [FIL, T], F32R, kind="ExternalInput")
    srep_d = nc.dram_tensor("srep", [N1, KF], F32, kind="ExternalInput")
    brep_d = nc.dram_tensor("brep", [N2, FC], F32R, kind="ExternalInput")
    cdef = [
        ("c_F2r", [N2, N2], F32R), ("c_F2i", [N2, N2], F32R), ("c_F2in", [N2, N2], F32R),
        ("c_Twr", [N2, N1], F32), ("c_Twi", [N2, N1], F32), ("c_Twin", [N2, N1], F32),
        ("c_M2", [N1s, N1s], F32R), ("c_M3", [N1s, N1s], BF16),
        ("c_L", [N2, 2 * N1 * N2], BF16),
        ("c_idr", [N2, N2], F32R), ("c_ones", [1, N2], F32R), ("c_idb", [N1s, N1s], BF16),
    ]
    cd = {}
    for name, shape, dt_ in cdef:
        cd[name] = nc.dram_tensor(name, shape, dt_, kind="ExternalInput")
    out_d = nc.dram_tensor("out", [BL, T, FC], F32, kind="ExternalOutput")
    dbg = {}
    if debug_dumps:
        for nm, shape in [("dBT", [N1s, RN]), ("dZ0A", [N1s, RN]), ("dHs", [N1s, KF]),
                          ("dG1", [N1s, KF]), ("dG2", [N1s, KF]), ("dDT0", [N2, 2 * N1 * FC])]:
            dbg[nm] = nc.dram_tensor(nm, shape, F32, kind="ExternalOutput")

    def chunks(total):
        return [(c0, min(total, c0 + MCH)) for c0 in range(0, total, MCH)]

    with tile.TileContext(nc) as tc:
        with tc.tile_pool(name="consts", bufs=1) as cpool, \
             tc.tile_pool(name="spec", bufs=1) as spool, \
             tc.tile_pool(name="gt", bufs=1) as gpool:
            ct = {}
            for name, shape, dt_ in cdef:
                t_ = cpool.tile(shape, dt_, tag=name)
                if name != "c_L":
                    nc.sync.dma_start(out=t_, in_=cd[name].ap())
                ct[name] = t_
            brep = cpool.tile([N2, FC], F32R, tag="brep")
            nc.sync.dma_start(out=brep, in_=brep_d.ap())
            srep = cpool.tile([N1, KF], F32, tag="srep")
            nc.sync.dma_start(out=srep, in_=srep_d.ap())

            Z0A = spool.tile([N1s, RN], BF16, tag="Z0A")   # [k1r;k1i | (row,k2)]
            Z0B = spool.tile([N1s, RN], BF16, tag="Z0B")   # [k1i;k1r | (row,k2)]
            G1 = gpool.tile([N1s, KF], BF16, tag="G1")     # [ Gr;Gr | (f,k2)]
            G2 = gpool.tile([N1s, KF], BF16, tag="G2")     # [-Gi;Gi | (f,k2)]
            from contextlib import ExitStack
            _fes = ExitStack()
            fwdbig = _fes.enter_context(tc.tile_pool(name="fwdbig", bufs=1))
            BT = fwdbig.tile([N1s, RN], F32R, tag="BT")    # [n1r;n1i | (row,k2)]
            BTH = fwdbig.tile([N1s, KF], F32R, tag="BTH")
            Hs = fwdbig.tile([N1s, KF], F32, tag="Hs")

            # ================= forward FFT of x rows =================
            with tc.tile_pool(name="fx", bufs=1) as fp, \
                 tc.tile_pool(name="fxp", bufs=1, space="PSUM") as fps, \
                 tc.tile_pool(name="t1p", bufs=2, space="PSUM") as t1ps:
                for b in range(BL):
                    xt = fp.tile([N2, N1 * C], F32R, tag=f"xt{b}")
                    nc.sync.dma_start(
                        out=xt, in_=xs_d.ap()[b].rearrange("(p q) c -> p (q c)", p=N2))
                    ps = fps.tile([N2, 2 * N1 * C], F32, tag=f"Aps{b}")
                    for comp, w in ((0, "c_F2r"), (1, "c_F2i")):
                        for c0, c1 in chunks(N1 * C):
                            nc.tensor.matmul(
                                ps[:, comp * N1 * C + c0: comp * N1 * C + c1],
                                ct[w], xt[:, c0:c1], start=True, stop=True)
                    # twiddle: Bq = A * W^(n1 k2); A free = (n1, c)
                    Ar = ps[:, :N1 * C].rearrange("p (n c) -> p n c", c=C)
                    Ai = ps[:, N1 * C:].rearrange("p (n c) -> p n c", c=C)
                    Bc = fp.tile([N2, 2 * N1 * C], F32R, tag=f"Bc{b}")
                    u = fp.tile([N2, N1 * C], F32, tag=f"u{b}")
                    v = fp.tile([N2, N1 * C], F32, tag=f"v{b}")

                    def bcx(w):
                        return ct[w][:, :, None].broadcast_to([N2, N1, C])

                    uv = u.rearrange("p (n c) -> p n c", c=C)
                    vv = v.rearrange("p (n c) -> p n c", c=C)
                    Brv = Bc[:, :N1 * C].rearrange("p (n c) -> p n c", c=C)
                    Biv = Bc[:, N1 * C:].rearrange("p (n c) -> p n c", c=C)
                    u2 = fp.tile([N2, N1 * C], F32, tag=f"u2{b}")
                    v2_ = fp.tile([N2, N1 * C], F32, tag=f"v2{b}")
                    u2v = u2.rearrange("p (n c) -> p n c", c=C)
                    v2v = v2_.rearrange("p (n c) -> p n c", c=C)
                    # gpsimd cannot read PSUM: stage A into SBUF via ACT for its half
                    Asb = fp.tile([N2, 2 * N1 * C], F32, tag=f"Asb{b}")
                    nc.scalar.copy(out=Asb, in_=ps)
                    Asr = Asb[:, :N1 * C].rearrange("p (n c) -> p n c", c=C)
                    Asi = Asb[:, N1 * C:].rearrange("p (n c) -> p n c", c=C)
                    nc.vector.tensor_tensor(out=uv, in0=Ar, in1=bcx("c_Twr"), op=AL.mult)
                    nc.vector.tensor_tensor(out=vv, in0=Ai, in1=bcx("c_Twin"), op=AL.mult)
                    nc.vector.tensor_tensor(out=Brv, in0=uv, in1=vv, op=AL.add)
                    nc.gpsimd.tensor_tensor(out=u2v, in0=Asr, in1=bcx("c_Twi"), op=AL.mult)
                    nc.gpsimd.tensor_tensor(out=v2v, in0=Asi, in1=bcx("c_Twr"), op=AL.mult)
                    nc.gpsimd.tensor_tensor(out=Biv, in0=u2v, in1=v2v, op=AL.add)
                    # T1: one fused transpose per row: [N2 | (comp,n1)] -> [(comp,n1) | N2]
                    Bview = Bc.rearrange("p (m n c) -> p m n c", m=2, c=C)
                    for c in range(C):
                        tp = t1ps.tile([N1s, N2], F32R, tag="t1")
                        nc.tensor.transpose(tp, Bview[:, :, :, c], ct["c_idr"])
                        row = b * C + c
                        nc.scalar.copy(out=BT[:, row * N2:(row + 1) * N2], in_=tp)

            # M2: Z0 = F1-stack^T @ BT
            with tc.tile_pool(name="m2p", bufs=1, space="PSUM") as m2ps:
                ps = m2ps.tile([N1s, RN], F32, tag="m2")
                for c0, c1 in chunks(RN):
                    nc.tensor.matmul(ps[:, c0:c1], ct["c_M2"], BT[:, c0:c1],
                                     start=True, stop=True)
                nc.vector.tensor_copy(out=Z0A, in_=ps)
            nc.sync.dma_start(out=Z0B[N1:, :], in_=Z0A[:N1, :])
            nc.sync.dma_start(out=Z0B[:N1, :], in_=Z0A[N1:, :])

            # ================= forward FFT of w rows (H), then G =================
            with tc.tile_pool(name="fh", bufs=1) as hp, \
                 tc.tile_pool(name="fhp", bufs=1, space="PSUM") as hps, \
                 tc.tile_pool(name="t1hp", bufs=2, space="PSUM") as t1hps:
                wtr = hp.tile([N2, FIL * N1], F32R, tag="wtr")
                wti = hp.tile([N2, FIL * N1], F32R, tag="wti")
                nc.sync.dma_start(out=wtr.rearrange("p (f n) -> p f n", f=FIL),
                                  in_=wr_d.ap().rearrange("f (p n) -> p f n", p=N2))
                nc.sync.dma_start(out=wti.rearrange("p (f n) -> p f n", f=FIL),
                                  in_=wi_d.ap().rearrange("f (p n) -> p f n", p=N2))
                ps = hps.tile([N2, 2 * FIL * N1], F32, tag="Hps")
                for c0, c1 in chunks(FIL * N1):
                    nc.tensor.matmul(ps[:, c0:c1], ct["c_F2r"], wtr[:, c0:c1],
                                     start=True, stop=False)
                    nc.tensor.matmul(ps[:, c0:c1], ct["c_F2in"], wti[:, c0:c1],
                                     start=False, stop=True)
                    d0 = FIL * N1
                    nc.tensor.matmul(ps[:, d0 + c0:d0 + c1], ct["c_F2i"], wtr[:, c0:c1],
                                     start=True, stop=False)
                    nc.tensor.matmul(ps[:, d0 + c0:d0 + c1], ct["c_F2r"], wti[:, c0:c1],
                                     start=False, stop=True)
                # twiddle; free = (f, n1), broadcast over f (outer)
                Ar = ps[:, :FIL * N1].rearrange("p (f n) -> p f n", f=FIL)
                Ai = ps[:, FIL * N1:].rearrange("p (f n) -> p f n", f=FIL)
                BHc = hp.tile([N2, FIL * 2 * N1], F32R, tag="BHc")
                u = hp.tile([N2, FIL * N1], F32, tag="uh")
                v = hp.tile([N2, FIL * N1], F32, tag="vh")

                def bch(w):
                    return ct[w][:, None, :].broadcast_to([N2, FIL, N1])

                uv = u.rearrange("p (f n) -> p f n", f=FIL)
                vv = v.rearrange("p (f n) -> p f n", f=FIL)
                BHv = BHc.rearrange("p (f m n) -> p f m n", f=FIL, m=2)
                nc.vector.tensor_tensor(out=uv, in0=Ar, in1=bch("c_Twr"), op=AL.mult)
                nc.vector.tensor_tensor(out=vv, in0=Ai, in1=bch("c_Twin"), op=AL.mult)
                nc.vector.tensor_tensor(out=BHv[:, :, 0, :], in0=uv, in1=vv, op=AL.add)
                nc.vector.tensor_tensor(out=uv, in0=Ar, in1=bch("c_Twi"), op=AL.mult)
                nc.vector.tensor_tensor(out=vv, in0=Ai, in1=bch("c_Twr"), op=AL.mult)
                nc.vector.tensor_tensor(out=BHv[:, :, 1, :], in0=uv, in1=vv, op=AL.add)
                for f in range(FIL):
                    tp = t1hps.tile([N1s, N2], F32R, tag="t1h")
                    nc.tensor.transpose(tp, BHc[:, f * 2 * N1:(f + 1) * 2 * N1], ct["c_idr"])
                    nc.scalar.copy(out=BTH[:, f * N2:(f + 1) * N2], in_=tp)

            with tc.tile_pool(name="m2hp", bufs=1, space="PSUM") as m2hps:
                ps = m2hps.tile([N1s, KF], F32, tag="m2h")
                for c0, c1 in chunks(KF):
                    nc.tensor.matmul(ps[:, c0:c1], ct["c_M2"], BTH[:, c0:c1],
                                     start=True, stop=True)
                nc.vector.tensor_copy(out=Hs, in_=ps)

            # G = conj(H) / (|H|^2 + s): all DVE ops at partition base 0;
            # cross-partition marshaling via SBUF->SBUF DMA.
            with tc.tile_pool(name="g", bufs=1) as gp:
                sq = gp.tile([N1s, KF], F32, tag="sq")
                nc.scalar.square(sq, Hs)
                sqB = gp.tile([N1, KF], F32, tag="sqB")
                HiB = gp.tile([N1, KF], F32, tag="HiB")
                nc.sync.dma_start(out=sqB, in_=sq[N1:, :])
                nc.sync.dma_start(out=HiB, in_=Hs[N1:, :])
                d = gp.tile([N1, KF], F32, tag="d")
                nc.vector.tensor_tensor(out=d, in0=sq[:N1, :], in1=sqB, op=AL.add)
                nc.vector.tensor_tensor(out=d, in0=d, in1=srep, op=AL.add)
                r = gp.tile([N1, KF], F32, tag="r")
                nc.vector.reciprocal(out=r, in_=d)
                rn = gp.tile([N1, KF], F32, tag="rn")
                nc.vector.tensor_scalar_mul(out=rn, in0=r, scalar1=-1.0)
                gtmp = gp.tile([N1, KF], BF16, tag="gtmp")
                nc.vector.tensor_tensor(out=G1[:N1, :], in0=Hs[:N1, :], in1=r, op=AL.mult)
                nc.vector.tensor_tensor(out=G2[:N1, :], in0=HiB, in1=r, op=AL.mult)
                nc.vector.tensor_tensor(out=gtmp, in0=HiB, in1=rn, op=AL.mult)
                nc.sync.dma_start(out=G1[N1:, :], in_=G1[:N1, :])
                nc.sync.dma_start(out=G2[N1:, :], in_=gtmp)

            if debug_dumps:
                F32b = F32
                nc.sync.dma_start(out=dbg["dBT"].ap(), in_=BT.bitcast(F32b))
                nc.gpsimd.dma_start(out=dbg["dZ0A"].ap(), in_=Z0A)
                nc.sync.dma_start(out=dbg["dHs"].ap(), in_=Hs)
                nc.gpsimd.dma_start(out=dbg["dG1"].ap(), in_=G1)
                nc.gpsimd.dma_start(out=dbg["dG2"].ap(), in_=G2)
            _fes.close()
            nc.sync.dma_start(out=ct["c_L"], in_=cd["c_L"].ap())
            # ================= inverse per (b, f) =================
            with tc.tile_pool(name="inv", bufs=2) as ip, \
                 tc.tile_pool(name="invs", bufs=2) as ip1, \
                 tc.tile_pool(name="dt", bufs=2) as dtp, \
                 tc.tile_pool(name="invp", bufs=2, space="PSUM") as ips, \
                 tc.tile_pool(name="t2p", bufs=2, space="PSUM") as t2ps, \
                 tc.tile_pool(name="yp", bufs=2, space="PSUM") as yps, \
                 tc.tile_pool(name="yev", bufs=3) as yp:
                for b in range(BL):
                    DT = dtp.tile([N2, 2 * N1 * FC], BF16, tag="DT")
                    # free layout: (fc, ri, n1) — DMA-transpose dest contiguous per (f,c)
                    dtm = DT.rearrange("p (fc ri n1) -> p ri n1 fc", fc=FC, ri=2, n1=N1)
                    for f in range(FIL):
                        zA = Z0A[:, b * KB:(b + 1) * KB].rearrange("p (c k) -> p c k", c=C)
                        zB = Z0B[:, b * KB:(b + 1) * KB].rearrange("p (c k) -> p c k", c=C)
                        g1 = G1[:, f * N2:(f + 1) * N2][:, None, :].broadcast_to([N1s, C, N2])
                        g2 = G2[:, f * N2:(f + 1) * N2][:, None, :].broadcast_to([N1s, C, N2])
                        veng = nc.gpsimd if (f % 8 == 2) else nc.vector
                        sfx = "g" if f % 3 == 2 else ""
                        zt1 = ip1.tile([N1s, KB], BF16, tag="zt1" + sfx)
                        zt2 = ip1.tile([N1s, KB], BF16, tag="zt2" + sfx)
                        zf = ip.tile([N1s, KB], BF16, tag="zf")
                        z1v = zt1.rearrange("p (c k) -> p c k", c=C)
                        z2v = zt2.rearrange("p (c k) -> p c k", c=C)
                        veng.tensor_tensor(out=z1v, in0=zA, in1=g1, op=AL.mult)
                        veng.tensor_tensor(out=z2v, in0=zB, in1=g2, op=AL.mult)
                        veng.tensor_tensor(out=zf, in0=zt1, in1=zt2, op=AL.add)
                        cps = ips.tile([N1s, KB], F32, tag="cps")
                        for c0, c1 in chunks(KB):
                            nc.tensor.matmul(cps[:, c0:c1], ct["c_M3"], zf[:, c0:c1],
                                             start=True, stop=True)
                        cs_ = ip.tile([N1s, KB], BF16, tag="cs")
                        nc.scalar.copy(out=cs_, in_=cps)  # ACT
                        # T2: PE transposes (bf16, 1cyc/row), one contiguous evac
                        tp = t2ps.tile([N2, C * N1s], BF16, tag="t2")
                        for c in range(C):
                            nc.tensor.transpose(
                                tp[:, c * N1s:(c + 1) * N1s],
                                cs_[:, c * N2:(c + 1) * N2], ct["c_idb"])
                        nc.scalar.copy(
                            out=DT[:, f * C * N1s:(f + 1) * C * N1s], in_=tp)
                    if debug_dumps and b == 0:
                        nc.gpsimd.dma_start(out=dbg["dDT0"].ap(), in_=DT)
                    # M4, batched 4 n1' per PSUM bank
                    NB = max(1, min(N1, MCH // FC))
                    for g0 in range(0, N1, NB):
                        gn = min(NB, N1 - g0)
                        ypsum = yps.tile([N2, NB * FC], F32, tag="yps")
                        for j in range(gn):
                            n1p = g0 + j
                            lr = ct["c_L"][:, n1p * N2:(n1p + 1) * N2]
                            li = ct["c_L"][:, (N1 + n1p) * N2:(N1 + n1p + 1) * N2]
                            rr = dtm[:, 0, n1p, :]
                            ri_ = dtm[:, 1, n1p, :]
                            sl = ypsum[:, j * FC:(j + 1) * FC]
                            nc.tensor.matmul(sl, lr, rr, start=(j == 0), stop=False)
                            nc.tensor.matmul(sl, li, ri_, start=False,
                                             stop=(j == gn - 1))
                        yt = yp.tile([N2, NB * FC], F32, tag="yt")
                        bb = brep[:, None, :].broadcast_to([N2, gn, FC])
                        nc.vector.tensor_tensor(
                            out=yt[:, :gn * FC].rearrange("p (j fc) -> p j fc", j=gn),
                            in0=ypsum[:, :gn * FC].rearrange("p (j fc) -> p j fc", j=gn),
                            in1=bb, op=AL.add)
                        nc.sync.dma_start(
                            out=out_d.ap()[b].rearrange("(n2 n1) fc -> n2 n1 fc", n1=N1)[:, g0:g0 + gn, :],
                            in_=yt[:, :gn * FC].rearrange("p (j fc) -> p j fc", j=gn))

    nc.compile()
    return nc


def host_inputs(cfg, x_sh, w_real, w_imag, s, b):
    """Build the per-core in_map (numpy) for one core's batch shard."""
    import ml_dtypes
    cs = host_consts(cfg)
    N1, N2, FIL, C, FC = cfg.N1, cfg.N2, cfg.FIL, cfg.C, cfg.FC
    m = {
        "xs": np.ascontiguousarray(x_sh, dtype=np.float32),
        "wr": np.ascontiguousarray(w_real, dtype=np.float32),
        "wi": np.ascontiguousarray(w_imag, dtype=np.float32),
        "srep": np.broadcast_to(s.reshape(1, FIL, 1), (N1, FIL, N2)).reshape(N1, FIL * N2).astype(np.float32).copy(),
        "brep": np.broadcast_to(b.reshape(1, FC), (N2, FC)).astype(np.float32).copy(),
    }
    cs["c_ones"] = np.ones((1, N2), dtype=np.float32)
    for k, v in cs.items():
        if k in ("c_L", "c_M3", "c_idb"):
            m[k] = v.astype(ml_dtypes.bfloat16)
        else:
            m[k] = v
    return m


_NC_CACHE = {}


def kernel(x, w_real, w_imag, s, b):
    """Full-input entry point: shard over 8 cores, run, gather."""
    from concourse.bass_utils import run_bass_kernel_spmd
    cfg = FULL
    n_cores = 8
    key = "full"
    if key not in _NC_CACHE:
        _NC_CACHE[key] = build_nc(cfg)
    nc = _NC_CACHE[key]
    x = np.asarray(x, dtype=np.float32)
    w_real = np.asarray(w_real, dtype=np.float32)
    w_imag = np.asarray(w_imag, dtype=np.float32)
    s = np.asarray(s, dtype=np.float32)
    b = np.asarray(b, dtype=np.float32)
    in_maps = []
    for i in range(n_cores):
        x_sh = x[i * cfg.BL:(i + 1) * cfg.BL]
        in_maps.append(host_inputs(cfg, x_sh, w_real, w_imag, s, b))
    res = run_bass_kernel_spmd(nc, in_maps, core_ids=list(range(n_cores)))
    outs = [res.results[i]["out"] for i in range(n_cores)]
    return np.concatenate(outs, axis=0).astype(np.float32)



# revision 4
# speedup vs baseline: 1.0212x; 1.0212x over previous
"""Trainium2 Bass kernel v5: Wiener deconvolution via 4-step CT FFT matmuls.

v5 over v4: H-twiddle split DVE(comp0,psum-direct)/Pool(comp1,ACT-staged);
x-twiddle on DVE psum-direct (no staging); PE warm-fill matmuls bridge the
forward gap so M2 runs at full p-state; finer PSUM pool lifetimes (Hps reuses
Ah's banks); a quarter of evac1s on DVE; per-half G broadcasts on SP.
"""
import sys

sys.path.insert(0, "/opt/trn_rl_repo")

import numpy as np


def _get_cc():
    import concourse.bacc as bacc
    import concourse.mybir as mybir
    import concourse.tile as tile
    return bacc, mybir, tile


class Cfg:
    def __init__(self, T=8192, N2=128, N1=64, BL=2, C=8, FIL=16):
        assert N1 * N2 == T
        self.T, self.N2, self.N1, self.BL, self.C, self.FIL = T, N2, N1, BL, C, FIL
        self.ROWS = BL * C
        self.FC = FIL * C


FULL = Cfg()

POOL_PAIRS = {(1, 3), (1, 7), (1, 11), (0, 3), (0, 7), (0, 11)}
EVAC1_DVE = lambda b, f: f in (2, 7, 12)


def host_consts(cfg):
    T, N1, N2 = cfg.T, cfg.N1, cfg.N2
    f32 = np.float32
    a2, a1 = np.arange(N2), np.arange(N1)
    cs = {}
    F2 = np.exp(-2j * np.pi * np.outer(a2, a2) / N2)          # [n2,k2]
    cs["blob_r"] = np.concatenate(
        [F2.real, F2.imag, -F2.imag], axis=1).astype(f32)
    Tw = np.exp(-2j * np.pi * np.outer(a2, a1) / T)           # [k2,n1]
    brep_placeholder = np.zeros((N2, cfg.FC), f32)
    cs["blob_f"] = np.concatenate(
        [Tw.real, Tw.imag, -Tw.imag, brep_placeholder], axis=1).astype(f32)
    F1 = np.exp(-2j * np.pi * np.outer(a1, a1) / N1)          # [n1,k1]
    M2 = np.hstack([np.vstack([F1.real, -F1.imag]),
                    np.vstack([F1.imag, F1.real])]).astype(f32)
    Fb1 = np.exp(2j * np.pi * np.outer(a1, a1) / N1)          # [k1,n1']
    M3 = np.hstack([np.vstack([Fb1.real, -Fb1.imag]),
                    np.vstack([Fb1.imag, Fb1.real])]).astype(f32)
    M3sw = np.vstack([-M3[N1:], M3[:N1]]).astype(f32)
    pad = np.ones((2 * N1, 1), f32)
    cs["blob_b"] = np.concatenate([M2, M3, M3sw, pad], axis=1).astype(f32)
    L = np.exp(2j * np.pi * (np.outer(a2, a2)[None, :, :] / N2
                             + (a1[:, None, None] * a2[None, :, None]) / T)) / T
    cL = np.empty((N2, 2, N1, N2), f32)                        # [k2, ri, n1', n2']
    cL[:, 0] = L.real.transpose(1, 0, 2)
    cL[:, 1] = -L.imag.transpose(1, 0, 2)
    cs["c_L"] = cL.reshape(N2, 2 * N1 * N2)
    return cs


def build_nc(cfg):
    bacc, mybir, tile = _get_cc()
    F32, F32R, BF16 = mybir.dt.float32, mybir.dt.float32r, mybir.dt.bfloat16
    AL = mybir.AluOpType
    T, N1, N2, BL, C, FIL = cfg.T, cfg.N2 * cfg.N1, cfg.N2, cfg.N1, cfg.BL, cfg.C
    # (re-bind clean)
    T, N2, N1, BL, C, FIL = cfg.T, cfg.N2, cfg.N1, cfg.BL, cfg.C, cfg.FIL
    FC = cfg.FC
    N1s = 2 * N1
    KF = FIL * N2
    MCH = 512
    HN = FIL * N1      # 1024
    XNb = N1 * C       # 512
    XN = BL * XNb      # 1024
    CK = C * N2        # 1024

    nc = bacc.Bacc("TRN2", debug=False)

    xs_d = nc.dram_tensor("xs", [N2, XN], F32R, kind="ExternalInput")
    wr_d = nc.dram_tensor("wr", [N2, HN], F32R, kind="ExternalInput")
    wi_d = nc.dram_tensor("wi", [N2, HN], F32R, kind="ExternalInput")
    s64_d = nc.dram_tensor("s64", [N1, FIL], F32, kind="ExternalInput")
    brow_d = nc.dram_tensor("brow", [1, FC], BF16, kind="ExternalInput")
    onesr_d = nc.dram_tensor("onesr", [1, N2], BF16, kind="ExternalInput")
    blob_r_d = nc.dram_tensor("blob_r", [N2, 3 * N2], F32R, kind="ExternalInput")
    blob_f_d = nc.dram_tensor("blob_f", [N2, 3 * N1 + FC], F32, kind="ExternalInput")
    blob_b_d = nc.dram_tensor("blob_b", [N1s, 3 * N1s + 1], BF16, kind="ExternalInput")
    cL_d = nc.dram_tensor("c_L", [N2, 2 * N1 * N2], BF16, kind="ExternalInput")
    out_d = nc.dram_tensor("out", [BL, T, FC], BF16, kind="ExternalOutput")

    def chunks(total):
        return [(c0, min(total, c0 + MCH)) for c0 in range(0, total, MCH)]

    with tile.TileContext(nc) as tc:
        from contextlib import ExitStack
        with tc.tile_pool(name="consts", bufs=1) as cpool, \
             tc.tile_pool(name="pers", bufs=1) as pers:
            blob_r = cpool.tile([N2, 3 * N2], F32R, tag="blob_r")
            nc.sync.dma_start(out=blob_r, in_=blob_r_d.ap())
            wtr = cpool.tile([N2, HN], F32R, tag="wtr")
            nc.sync.dma_start(out=wtr, in_=wr_d.ap())
            wti = cpool.tile([N2, HN], F32R, tag="wti")
            nc.sync.dma_start(out=wti, in_=wi_d.ap())
            xt = cpool.tile([N2, XN], F32R, tag="xt")
            nc.sync.dma_start(out=xt, in_=xs_d.ap())
            blob_f = cpool.tile([N2, 3 * N1 + FC], F32, tag="blob_f")
            nc.sync.dma_start(out=blob_f, in_=blob_f_d.ap())
            blob_b = cpool.tile([N1s, 3 * N1s + 1], BF16, tag="blob_b")
            nc.sync.dma_start(out=blob_b, in_=blob_b_d.ap())
            s64 = cpool.tile([N1, FIL], F32, tag="s64")
            nc.sync.dma_start(out=s64, in_=s64_d.ap())
            brow = cpool.tile([1, FC], BF16, tag="brow")
            nc.sync.dma_start(out=brow, in_=brow_d.ap())
            onesr = cpool.tile([1, N2], BF16, tag="onesr")
            nc.sync.dma_start(out=onesr, in_=onesr_d.ap())
            cL = cpool.tile([N2, 2 * N1 * N2], BF16, tag="cL")
            nc.sync.dma_start(out=cL, in_=cL_d.ap())

            F2r = blob_r[:, 0:N2]
            F2i = blob_r[:, N2:2 * N2]
            F2in = blob_r[:, 2 * N2:3 * N2]
            Twr = blob_f[:, 0:N1]
            Twi = blob_f[:, N1:2 * N1]
            Twin = blob_f[:, 2 * N1:3 * N1]
            brep = blob_f[:, 3 * N1:3 * N1 + FC]
            cM2 = blob_b[:, 0:N1s]
            cM3 = blob_b[:, N1s:2 * N1s]
            cM3sw = blob_b[:, 2 * N1s:3 * N1s]

            Z0A = pers.tile([N1s, XN * 2], BF16, tag="Z0A")        # [k1s | (b,c,k2)]
            sqB = pers.tile([N1, FIL * N2], F32, tag="sqB")
            nc.scalar.copy(out=sqB.rearrange("p (f q) -> p f q", f=FIL),
                           in_=s64[:, :, None].broadcast_to([N1, FIL, N2]))
            G1 = pers.tile([N1s, KF], BF16, tag="G1")
            G2 = pers.tile([N1s, KF], BF16, tag="G2")

            fes = ExitStack()
            fwd = fes.enter_context(tc.tile_pool(name="fwd", bufs=1))
            pxes = ExitStack()
            pAx = pxes.enter_context(tc.tile_pool(name="pAx", bufs=1, space="PSUM"))
            phes = ExitStack()
            pAh = phes.enter_context(tc.tile_pool(name="pAh", bufs=1, space="PSUM"))

            # ---------- M1 H ----------
            Ah = pAh.tile([N2, 2 * HN], F32, tag="Ah")              # [k2 | (comp,f,n1)]
            for c0, c1 in chunks(HN):
                nc.tensor.matmul(Ah[:, c0:c1], F2r, wtr[:, c0:c1], start=True, stop=False)
                nc.tensor.matmul(Ah[:, c0:c1], F2in, wti[:, c0:c1], start=False, stop=True)
                nc.tensor.matmul(Ah[:, HN + c0:HN + c1], F2i, wtr[:, c0:c1],
                                 start=True, stop=False)
                nc.tensor.matmul(Ah[:, HN + c0:HN + c1], F2r, wti[:, c0:c1],
                                 start=False, stop=True)

            # ---------- M1 x ----------
            Ax = pAx.tile([N2, 2 * XN], F32, tag="Ax")              # [k2 | (comp,b,n1,c)]
            for c0, c1 in chunks(XN):
                nc.tensor.matmul(Ax[:, c0:c1], F2r, xt[:, c0:c1], start=True, stop=True)
                nc.tensor.matmul(Ax[:, XN + c0:XN + c1], F2i, xt[:, c0:c1],
                                 start=True, stop=True)

            # ---------- H twiddle: comp0 on DVE (psum-direct), comp1 on Pool ----------
            Bh = fwd.tile([N2, FIL * 2 * N1], BF16, tag="Bh")      # [k2 | (f,comp,n1)]
            Bhv = Bh.rearrange("p (f m n) -> p f m n", f=FIL, m=2)
            Ahr = Ah[:, :HN].rearrange("p (f n) -> p f n", f=FIL)
            Ahi = Ah[:, HN:].rearrange("p (f n) -> p f n", f=FIL)
            # stage Ah to SBUF for the Pool half
            Ahs = fwd.tile([N2, 2 * HN], F32, tag="Ahs")
            nc.scalar.copy(out=Ahs[:, :HN], in_=Ah[:, :HN])
            nc.scalar.copy(out=Ahs[:, HN:], in_=Ah[:, HN:])
            Asr = Ahs[:, :HN].rearrange("p (f n) -> p f n", f=FIL)
            Asi = Ahs[:, HN:].rearrange("p (f n) -> p f n", f=FIL)
            uh = fwd.tile([N2, HN], F32, tag="uh")
            vh = fwd.tile([N2, HN], F32, tag="vh")
            uhv = uh.rearrange("p (f n) -> p f n", f=FIL)
            vhv = vh.rearrange("p (f n) -> p f n", f=FIL)
            uh2 = fwd.tile([N2, HN], F32, tag="uh2")
            vh2 = fwd.tile([N2, HN], F32, tag="vh2")
            uh2v = uh2.rearrange("p (f n) -> p f n", f=FIL)
            vh2v = vh2.rearrange("p (f n) -> p f n", f=FIL)

            def bch(w):
                return w[:, None, :].broadcast_to([N2, FIL, N1])

            nc.vector.tensor_tensor(out=uhv, in0=Ahr, in1=bch(Twr), op=AL.mult)
            nc.vector.tensor_tensor(out=vhv, in0=Ahi, in1=bch(Twin), op=AL.mult)
            nc.vector.tensor_tensor(out=Bhv[:, :, 0, :], in0=uhv, in1=vhv, op=AL.add)
            nc.gpsimd.tensor_tensor(out=uh2v, in0=Asr, in1=bch(Twi), op=AL.mult)
            nc.gpsimd.tensor_tensor(out=vh2v, in0=Asi, in1=bch(Twr), op=AL.mult)
            nc.gpsimd.tensor_tensor(out=Bhv[:, :, 1, :], in0=uh2v, in1=vh2v, op=AL.add)

            # ---------- x twiddle on DVE (psum-direct), b1 then b0 ----------
            Bc = fwd.tile([N2, BL * C * 2 * N1], BF16, tag="Bc")   # [k2|(b,c,comp,n1)]
            Bcv = Bc.rearrange("p (b c m n) -> p b c m n", b=BL, c=C, m=2)
            Axr = Ax[:, :XN].rearrange("p (b n c) -> p b c n", b=BL, c=C)
            Axi = Ax[:, XN:].rearrange("p (b n c) -> p b c n", b=BL, c=C)
            ux = fwd.tile([N2, XNb], F32, tag="ux")
            vx = fwd.tile([N2, XNb], F32, tag="vx")
            uxv = ux.rearrange("p (c n) -> p c n", c=C)
            vxv = vx.rearrange("p (c n) -> p c n", c=C)

            def bcx(w):
                return w[:, None, :].broadcast_to([N2, C, N1])

            for b in (1, 0):
                nc.vector.tensor_tensor(out=uxv, in0=Axr[:, b], in1=bcx(Twr), op=AL.mult)
                nc.vector.tensor_tensor(out=vxv, in0=Axi[:, b], in1=bcx(Twin), op=AL.mult)
                nc.vector.tensor_tensor(out=Bcv[:, b, :, 0, :], in0=uxv, in1=vxv, op=AL.add)
                nc.vector.tensor_tensor(out=uxv, in0=Axr[:, b], in1=bcx(Twi), op=AL.mult)
                nc.vector.tensor_tensor(out=vxv, in0=Axi[:, b], in1=bcx(Twr), op=AL.mult)
                nc.vector.tensor_tensor(out=Bcv[:, b, :, 1, :], in0=uxv, in1=vxv, op=AL.add)

            # ---------- T1 block transposes (DMA xbar) ----------
            BTH = fwd.tile([N1s, KF], BF16, tag="BTH")             # [(comp n1)|(f,k2)]
            nc.sync.dma_start_transpose(
                out=BTH.rearrange("p (f q) -> p f q", f=FIL), in_=Bh)
            BT = fwd.tile([N1s, XN * 2], BF16, tag="BT")           # [(comp n1)|(b,c,k2)]
            BTv = BT.rearrange("p (b c q) -> p b c q", b=BL, c=C)
            nc.sync.dma_start_transpose(out=BTv[:, 1], in_=Bc[:, CK:])
            nc.sync.dma_start_transpose(out=BTv[:, 0], in_=Bc[:, :CK])

            # ---------- M2h (Hps reuses Ah's banks only) ----------
            phes.close()
            pHes = ExitStack()
            pH = pHes.enter_context(tc.tile_pool(name="pH", bufs=1, space="PSUM"))
            Hps = pH.tile([N1s, KF], F32, tag="Hps")
            for c0, c1 in chunks(KF):
                nc.tensor.matmul(Hps[:, c0:c1], cM2, BTH[:, c0:c1], start=True, stop=True)
            Hs = pers.tile([N1s, KF], F32, tag="Hs")
            sq = pers.tile([N1s, KF], F32, tag="sq")
            HiB = pers.tile([N1, KF], F32, tag="HiB")
            QW = KF // 4
            for q in range(4):
                qs = slice(q * QW, (q + 1) * QW)
                nc.scalar.copy(out=Hs[:, qs], in_=Hps[:, qs])
                nc.scalar.dma_start(out=HiB[:, qs], in_=Hs[N1:, qs])
                nc.scalar.square(sq[:, qs], Hs[:, qs])
                nc.gpsimd.dma_start(out=sqB[:, qs], in_=sq[N1:, qs], accum_op=AL.add)

            # ---------- M2x ----------
            pHes.close()
            pxes.close()
            pZes = ExitStack()
            pZ = pZes.enter_context(tc.tile_pool(name="pZ", bufs=1, space="PSUM"))
            Zps = pZ.tile([N1s, XN * 2], F32, tag="Zps")           # [k1s | (b,c,k2)]
            for b in (1, 0):
                for c0, c1 in chunks(CK):
                    nc.tensor.matmul(Zps[:, b * CK + c0:b * CK + c1], cM2,
                                     BT[:, b * CK + c0:b * CK + c1], start=True, stop=True)
                nc.scalar.copy(out=Z0A[:, b * CK:(b + 1) * CK],
                               in_=Zps[:, b * CK:(b + 1) * CK])

            # ---------- G, pipelined per f-quarter (DVE); d in-place in sqB ----------
            def g_quarter(q):
                qs = slice(q * QW, (q + 1) * QW)
                nc.vector.tensor_tensor(out=sqB[:, qs], in0=sq[:N1, qs],
                                        in1=sqB[:, qs], op=AL.add)
                nc.vector.reciprocal(out=sq[:N1, qs], in_=sqB[:, qs])
                nc.vector.tensor_tensor(out=G1[:N1, qs], in0=Hs[:N1, qs],
                                        in1=sq[:N1, qs], op=AL.mult)
                nc.vector.tensor_tensor(out=G2[:N1, qs], in0=HiB[:, qs],
                                        in1=sq[:N1, qs], op=AL.mult)
                nc.sync.dma_start(out=G1[N1:, qs], in_=G1[:N1, qs])
                nc.sync.dma_start(out=G2[N1:, qs], in_=G2[:N1, qs])

            g_quarter(0)

            pZes.close()
            fes.close()

            # ================= inverse =================
            with tc.tile_pool(name="dt", bufs=1) as dtp, \
                 tc.tile_pool(name="stg", bufs=1) as stp, \
                 tc.tile_pool(name="zt", bufs=3) as ztp, \
                 tc.tile_pool(name="cse", bufs=3) as csp, \
                 tc.tile_pool(name="ddp", bufs=2, space="PSUM") as ddp, \
                 tc.tile_pool(name="yp", bufs=4, space="PSUM") as yps:
                DT0 = dtp.tile([N2, N1s * FIL * C], BF16, tag="DT0")
                DT1 = dtp.tile([N2, N1s * FIL * C], BF16, tag="DT1")
                DT = [DT0, DT1]                                    # [k2 | (f,c,n1s')]
                STG0 = stp.tile([N2, N1 * FC], BF16, tag="STG0")
                STG1 = stp.tile([N2, N1 * FC], BF16, tag="STG1")
                STG = [STG0, STG1]                                 # [n2' | (n1',fc)]
                cLv = cL.rearrange("p (m n q) -> p m n q", m=2, n=N1)
                zvA = Z0A.rearrange("p (b c q) -> p b c q", b=BL, c=C)

                def cmul_m3_t2(b, f):
                    eng = nc.gpsimd if (b, f) in POOL_PAIRS else nc.vector
                    g1 = G1[:, f * N2:(f + 1) * N2][:, None, :].broadcast_to([N1s, C, N2])
                    g2 = G2[:, f * N2:(f + 1) * N2][:, None, :].broadcast_to([N1s, C, N2])
                    zt1 = ztp.tile([N1s, CK], BF16, tag="zt1")
                    zt2 = ztp.tile([N1s, CK], BF16, tag="zt2")
                    eng.tensor_tensor(out=zt1.rearrange("p (c q) -> p c q", c=C),
                                      in0=zvA[:, b], in1=g1, op=AL.mult)
                    eng.tensor_tensor(out=zt2.rearrange("p (c q) -> p c q", c=C),
                                      in0=zvA[:, b], in1=g2, op=AL.mult)
                    DD = ddp.tile([N1s, CK], F32, tag="DD")
                    for c0, c1 in chunks(CK):
                        nc.tensor.matmul(DD[:, c0:c1], cM3, zt1[:, c0:c1],
                                         start=True, stop=False)
                        nc.tensor.matmul(DD[:, c0:c1], cM3sw, zt2[:, c0:c1],
                                         start=False, stop=True)
                    cse = csp.tile([N1s, CK], BF16, tag="cse")
                    if EVAC1_DVE(b, f):
                        nc.vector.tensor_copy(out=cse, in_=DD)
                    else:
                        nc.scalar.copy(out=cse, in_=DD)
                    dtv = DT[b].rearrange("p (fi c n) -> p fi c n",
                                          n=N1s, fi=FIL, c=C)[:, f]
                    nc.sync.dma_start_transpose(out=dtv, in_=cse)

                def m4_mm(b, g0, gn=4, seeded=True):
                    dtm = DT[b].rearrange("p (fi c n) -> p n fi c", n=N1s, fi=FIL, c=C)
                    ypsum = yps.tile([N2, gn * FC], F32, tag="yps")
                    for j in range(gn):
                        n1p = g0 + j
                        sl = ypsum[:, j * FC:(j + 1) * FC]
                        if seeded:
                            nc.tensor.matmul(sl, onesr, brow, start=True, stop=False)
                        nc.tensor.matmul(sl, cLv[:, 0, n1p, :], dtm[:, n1p],
                                         start=not seeded, stop=False)
                        nc.tensor.matmul(sl, cLv[:, 1, n1p, :], dtm[:, N1 + n1p],
                                         start=False, stop=True)
                    return ypsum

                def m4_evac(b, g0, ypsum, gn=4, eng="act"):
                    dst = STG[b][:, g0 * FC:(g0 + gn) * FC]
                    if eng == "act":
                        nc.scalar.copy(out=dst, in_=ypsum)
                    else:
                        bb = brep[:, None, :].broadcast_to([N2, gn, FC])
                        nc.vector.tensor_tensor(
                            out=dst.rearrange("p (j fc) -> p j fc", j=gn),
                            in0=ypsum.rearrange("p (j fc) -> p j fc", j=gn),
                            in1=bb, op=AL.add)

                def m4_group(b, g0, gn=4, eng="act"):
                    seeded = eng == "act"
                    m4_evac(b, g0, m4_mm(b, g0, gn, seeded), gn, eng)

                def out_chunk(b, g0, gn=16):
                    nc.scalar.dma_start(
                        out=out_d.ap()[b].rearrange(
                            "(q n) fc -> q (n fc)", n=N1)[:, g0 * FC:(g0 + gn) * FC],
                        in_=STG[b][:, g0 * FC:(g0 + gn) * FC])

                for f in range(FIL):
                    if f in (1, 4, 7):
                        g_quarter(f // 3 + 1)
                    cmul_m3_t2(1, f)
                pend = []
                done1 = 0
                for f in range(FIL):
                    cmul_m3_t2(0, f)
                    # retire deferred b1 evacs (emitted 2 pairs after their MMs)
                    while pend and pend[0][0] <= f - 2:
                        _, g0, yp_t, eng = pend.pop(0)
                        m4_evac(1, g0, yp_t, 4, eng)
                        done1 += 1
                        if done1 == 8:
                            out_chunk(1, 0)
                        elif done1 == 12:
                            out_chunk(1, 16)
                    if f >= 8:
                        for g in range(3):
                            gi = (f - 8) * 3 + g
                            if gi < 16:
                                eng = "dve" if gi % 2 == 0 else "act"
                                yp_t = m4_mm(1, gi * 4, 4, seeded=(eng == "act"))
                                pend.append((f, gi * 4, yp_t, eng))
                for _, g0, yp_t, eng in pend:
                    m4_evac(1, g0, yp_t, 4, eng)
                out_chunk(1, 32)
                out_chunk(1, 48)
                for g0 in range(0, N1, 4):
                    m4_group(0, g0)
                    if g0 % 16 == 12:
                        out_chunk(0, g0 - 12)

    nc.compile()
    return nc


def host_inputs(cfg, x_sh, w_real, w_imag, s, b):
    """Build the per-core in_map (numpy) for one core's batch shard."""
    import ml_dtypes
    cs = host_consts(cfg)
    N1, N2, FIL, C, FC, BL = cfg.N1, cfg.N2, cfg.FIL, cfg.C, cfg.FC, cfg.BL
    f32 = np.float32
    x_sh = np.asarray(x_sh, f32)
    xs = np.ascontiguousarray(
        x_sh.reshape(BL, N2, N1, C).transpose(1, 0, 2, 3)).reshape(N2, BL * N1 * C)
    wr = np.ascontiguousarray(
        np.asarray(w_real, f32).reshape(FIL, N2, N1).transpose(1, 0, 2)).reshape(N2, FIL * N1)
    wi = np.ascontiguousarray(
        np.asarray(w_imag, f32).reshape(FIL, N2, N1).transpose(1, 0, 2)).reshape(N2, FIL * N1)
    blob_f = cs["blob_f"].copy()
    blob_f[:, 3 * N1:] = np.broadcast_to(np.asarray(b, f32).reshape(1, FC), (N2, FC))
    return {
        "xs": xs, "wr": wr, "wi": wi,
        "s64": np.broadcast_to(np.asarray(s, f32).reshape(1, FIL), (N1, FIL)).copy(),
        "brow": np.asarray(b, f32).reshape(1, FC).astype(ml_dtypes.bfloat16),
        "onesr": np.ones((1, N2), f32).astype(ml_dtypes.bfloat16),
        "blob_r": cs["blob_r"],
        "blob_f": blob_f,
        "blob_b": cs["blob_b"].astype(ml_dtypes.bfloat16),
        "c_L": cs["c_L"].astype(ml_dtypes.bfloat16),
    }


_NC_CACHE = {}


def kernel(x, w_real, w_imag, s, b):
    """Full-input entry point: shard over 8 cores, run, gather."""
    from concourse.bass_utils import run_bass_kernel_spmd
    cfg = FULL
    n_cores = 8
    if "full" not in _NC_CACHE:
        _NC_CACHE["full"] = build_nc(cfg)
    nc = _NC_CACHE["full"]
    x = np.asarray(x, dtype=np.float32)
    in_maps = [host_inputs(cfg, x[i * cfg.BL:(i + 1) * cfg.BL], w_real, w_imag, s, b)
               for i in range(n_cores)]
    res = run_bass_kernel_spmd(nc, in_maps, core_ids=list(range(n_cores)))
    outs = [np.asarray(res.results[i]["out"]).astype(np.float32) for i in range(n_cores)]
    return np.concatenate(outs, axis=0)


# revision 6
# speedup vs baseline: 1.0816x; 1.0591x over previous
"""Trainium2 Bass kernel v5: Wiener deconvolution via 4-step CT FFT matmuls.

v5 over v4: H-twiddle split DVE(comp0,psum-direct)/Pool(comp1,ACT-staged);
x-twiddle on DVE psum-direct (no staging); PE warm-fill matmuls bridge the
forward gap so M2 runs at full p-state; finer PSUM pool lifetimes (Hps reuses
Ah's banks); a quarter of evac1s on DVE; per-half G broadcasts on SP.
"""
import sys

sys.path.insert(0, "/opt/trn_rl_repo")

import numpy as np


def _get_cc():
    import concourse.bacc as bacc
    import concourse.mybir as mybir
    import concourse.tile as tile
    return bacc, mybir, tile


class Cfg:
    def __init__(self, T=8192, N2=128, N1=64, BL=2, C=8, FIL=16):
        assert N1 * N2 == T
        self.T, self.N2, self.N1, self.BL, self.C, self.FIL = T, N2, N1, BL, C, FIL
        self.ROWS = BL * C
        self.FC = FIL * C


FULL = Cfg()

POOL_PAIRS = {(1, 4), (1, 8), (1, 12), (0, 4), (0, 8), (0, 12)}
EVAC1_DVE = lambda b, f: f in (2, 7, 12)


def host_consts(cfg):
    T, N1, N2 = cfg.T, cfg.N1, cfg.N2
    f32 = np.float32
    a2, a1 = np.arange(N2), np.arange(N1)
    cs = {}
    F2 = np.exp(-2j * np.pi * np.outer(a2, a2) / N2)          # [n2,k2]
    cs["blob_r"] = np.concatenate(
        [F2.real, F2.imag, -F2.imag], axis=1).astype(f32)
    Tw = np.exp(-2j * np.pi * np.outer(a2, a1) / T)           # [k2,n1]
    brep_placeholder = np.zeros((N2, cfg.FC), f32)
    cs["blob_f"] = np.concatenate(
        [Tw.real, Tw.imag, -Tw.imag, brep_placeholder], axis=1).astype(f32)
    F1 = np.exp(-2j * np.pi * np.outer(a1, a1) / N1)          # [n1,k1]
    M2 = np.hstack([np.vstack([F1.real, -F1.imag]),
                    np.vstack([F1.imag, F1.real])]).astype(f32)
    Fb1 = np.exp(2j * np.pi * np.outer(a1, a1) / N1)          # [k1,n1']
    M3 = np.hstack([np.vstack([Fb1.real, -Fb1.imag]),
                    np.vstack([Fb1.imag, Fb1.real])]).astype(f32)
    M3sw = np.vstack([-M3[N1:], M3[:N1]]).astype(f32)
    pad = np.ones((2 * N1, 1), f32)
    cs["blob_b"] = np.concatenate([M2, M3, M3sw, pad], axis=1).astype(f32)
    L = np.exp(2j * np.pi * (np.outer(a2, a2)[None, :, :] / N2
                             + (a1[:, None, None] * a2[None, :, None]) / T)) / T
    cL = np.empty((N2, 2, N1, N2), f32)                        # [k2, ri, n1', n2']
    cL[:, 0] = L.real.transpose(1, 0, 2)
    cL[:, 1] = -L.imag.transpose(1, 0, 2)
    cs["c_L"] = cL.reshape(N2, 2 * N1 * N2)
    return cs


def build_nc(cfg):
    bacc, mybir, tile = _get_cc()
    F32, F32R, BF16 = mybir.dt.float32, mybir.dt.float32r, mybir.dt.bfloat16
    AL = mybir.AluOpType
    T, N1, N2, BL, C, FIL = cfg.T, cfg.N2 * cfg.N1, cfg.N2, cfg.N1, cfg.BL, cfg.C
    # (re-bind clean)
    T, N2, N1, BL, C, FIL = cfg.T, cfg.N2, cfg.N1, cfg.BL, cfg.C, cfg.FIL
    FC = cfg.FC
    N1s = 2 * N1
    KF = FIL * N2
    MCH = 512
    HN = FIL * N1      # 1024
    XNb = N1 * C       # 512
    XN = BL * XNb      # 1024
    CK = C * N2        # 1024

    nc = bacc.Bacc("TRN2", debug=False)

    xs_d = nc.dram_tensor("xs", [N2, XN], F32R, kind="ExternalInput")
    wr_d = nc.dram_tensor("wr", [N2, HN], F32R, kind="ExternalInput")
    wi_d = nc.dram_tensor("wi", [N2, HN], F32R, kind="ExternalInput")
    s64_d = nc.dram_tensor("s64", [N1, FIL], F32, kind="ExternalInput")
    brow_d = nc.dram_tensor("brow", [1, FC], BF16, kind="ExternalInput")
    onesr_d = nc.dram_tensor("onesr", [1, N2], BF16, kind="ExternalInput")
    blob_r_d = nc.dram_tensor("blob_r", [N2, 3 * N2], F32R, kind="ExternalInput")
    blob_f_d = nc.dram_tensor("blob_f", [N2, 3 * N1 + FC], F32, kind="ExternalInput")
    blob_b_d = nc.dram_tensor("blob_b", [N1s, 3 * N1s + 1], BF16, kind="ExternalInput")
    cL_d = nc.dram_tensor("c_L", [N2, 2 * N1 * N2], BF16, kind="ExternalInput")
    out_d = nc.dram_tensor("out", [BL, T, FC], BF16, kind="ExternalOutput")

    def chunks(total):
        return [(c0, min(total, c0 + MCH)) for c0 in range(0, total, MCH)]

    with tile.TileContext(nc) as tc:
        from contextlib import ExitStack
        with tc.tile_pool(name="consts", bufs=1) as cpool, \
             tc.tile_pool(name="pers", bufs=1) as pers:
            blob_r = cpool.tile([N2, 3 * N2], F32R, tag="blob_r")
            nc.sync.dma_start(out=blob_r, in_=blob_r_d.ap())
            wtr = cpool.tile([N2, HN], F32R, tag="wtr")
            nc.sync.dma_start(out=wtr, in_=wr_d.ap())
            wti = cpool.tile([N2, HN], F32R, tag="wti")
            nc.sync.dma_start(out=wti, in_=wi_d.ap())
            xt = cpool.tile([N2, XN], F32R, tag="xt")
            nc.sync.dma_start(out=xt, in_=xs_d.ap())
            blob_f = cpool.tile([N2, 3 * N1 + FC], F32, tag="blob_f")
            nc.sync.dma_start(out=blob_f, in_=blob_f_d.ap())
            blob_b = cpool.tile([N1s, 3 * N1s + 1], BF16, tag="blob_b")
            nc.sync.dma_start(out=blob_b, in_=blob_b_d.ap())
            s64 = cpool.tile([N1, FIL], F32, tag="s64")
            nc.sync.dma_start(out=s64, in_=s64_d.ap())
            brow = cpool.tile([1, FC], BF16, tag="brow")
            nc.sync.dma_start(out=brow, in_=brow_d.ap())
            onesr = cpool.tile([1, N2], BF16, tag="onesr")
            nc.sync.dma_start(out=onesr, in_=onesr_d.ap())
            cL = cpool.tile([N2, 2 * N1 * N2], BF16, tag="cL")
            nc.sync.dma_start(out=cL, in_=cL_d.ap())

            F2r = blob_r[:, 0:N2]
            F2i = blob_r[:, N2:2 * N2]
            F2in = blob_r[:, 2 * N2:3 * N2]
            Twr = blob_f[:, 0:N1]
            Twi = blob_f[:, N1:2 * N1]
            Twin = blob_f[:, 2 * N1:3 * N1]
            brep = blob_f[:, 3 * N1:3 * N1 + FC]
            cM2 = blob_b[:, 0:N1s]
            cM3 = blob_b[:, N1s:2 * N1s]
            cM3sw = blob_b[:, 2 * N1s:3 * N1s]

            Z0A = pers.tile([N1s, XN * 2], BF16, tag="Z0A")        # [k1s | (b,c,k2)]
            sqB = pers.tile([N1, FIL * N2], F32, tag="sqB")
            G1 = pers.tile([N1s, KF], BF16, tag="G1")
            G2 = pers.tile([N1s, KF], BF16, tag="G2")

            fes = ExitStack()
            fwd = fes.enter_context(tc.tile_pool(name="fwd", bufs=1))
            pxes = ExitStack()
            pAx = pxes.enter_context(tc.tile_pool(name="pAx", bufs=1, space="PSUM"))
            phes = ExitStack()
            pAh = phes.enter_context(tc.tile_pool(name="pAh", bufs=1, space="PSUM"))

            # ---------- M1 H ----------
            Ah = pAh.tile([N2, 2 * HN], F32, tag="Ah")              # [k2 | (comp,f,n1)]
            for c0, c1 in chunks(HN):
                nc.tensor.matmul(Ah[:, c0:c1], F2r, wtr[:, c0:c1], start=True, stop=False)
                nc.tensor.matmul(Ah[:, c0:c1], F2in, wti[:, c0:c1], start=False, stop=True)
                nc.tensor.matmul(Ah[:, HN + c0:HN + c1], F2i, wtr[:, c0:c1],
                                 start=True, stop=False)
                nc.tensor.matmul(Ah[:, HN + c0:HN + c1], F2r, wti[:, c0:c1],
                                 start=False, stop=True)

            # ---------- M1 x ----------
            Ax = pAx.tile([N2, 2 * XN], F32, tag="Ax")              # [k2 | (comp,b,n1,c)]
            for c0, c1 in chunks(XN):
                nc.tensor.matmul(Ax[:, c0:c1], F2r, xt[:, c0:c1], start=True, stop=True)
                nc.tensor.matmul(Ax[:, XN + c0:XN + c1], F2i, xt[:, c0:c1],
                                 start=True, stop=True)

            # ---------- H twiddle: comp0 on DVE (psum-direct), comp1 on Pool ----------
            Bh = fwd.tile([N2, FIL * 2 * N1], BF16, tag="Bh")      # [k2 | (f,comp,n1)]
            Bhv = Bh.rearrange("p (f m n) -> p f m n", f=FIL, m=2)
            Ahr = Ah[:, :HN].rearrange("p (f n) -> p f n", f=FIL)
            Ahi = Ah[:, HN:].rearrange("p (f n) -> p f n", f=FIL)
            # stage Ah to SBUF for the Pool half
            Ahs = fwd.tile([N2, 2 * HN], F32, tag="Ahs")
            nc.scalar.copy(out=Ahs[:, :HN], in_=Ah[:, :HN])
            nc.scalar.copy(out=Ahs[:, HN:], in_=Ah[:, HN:])
            Asr = Ahs[:, :HN].rearrange("p (f n) -> p f n", f=FIL)
            Asi = Ahs[:, HN:].rearrange("p (f n) -> p f n", f=FIL)
            uh = fwd.tile([N2, HN], F32, tag="uh")
            vh = fwd.tile([N2, HN], F32, tag="vh")
            uhv = uh.rearrange("p (f n) -> p f n", f=FIL)
            vhv = vh.rearrange("p (f n) -> p f n", f=FIL)
            uh2 = fwd.tile([N2, HN], F32, tag="uh2")
            vh2 = fwd.tile([N2, HN], F32, tag="vh2")
            uh2v = uh2.rearrange("p (f n) -> p f n", f=FIL)
            vh2v = vh2.rearrange("p (f n) -> p f n", f=FIL)

            def bch(w):
                return w[:, None, :].broadcast_to([N2, FIL, N1])

            nc.vector.tensor_tensor(out=uhv, in0=Ahr, in1=bch(Twr), op=AL.mult)
            nc.vector.tensor_tensor(out=vhv, in0=Ahi, in1=bch(Twin), op=AL.mult)
            nc.vector.tensor_tensor(out=Bhv[:, :, 0, :], in0=uhv, in1=vhv, op=AL.add)
            nc.gpsimd.tensor_tensor(out=uh2v, in0=Asr, in1=bch(Twi), op=AL.mult)
            nc.gpsimd.tensor_tensor(out=vh2v, in0=Asi, in1=bch(Twr), op=AL.mult)
            nc.gpsimd.tensor_tensor(out=Bhv[:, :, 1, :], in0=uh2v, in1=vh2v, op=AL.add)

            # ---------- x twiddle on DVE (psum-direct), b1 then b0 ----------
            Bc = fwd.tile([N2, BL * C * 2 * N1], BF16, tag="Bc")   # [k2|(b,c,comp,n1)]
            Bcv = Bc.rearrange("p (b c m n) -> p b c m n", b=BL, c=C, m=2)
            Axr = Ax[:, :XN].rearrange("p (b n c) -> p b c n", b=BL, c=C)
            Axi = Ax[:, XN:].rearrange("p (b n c) -> p b c n", b=BL, c=C)
            ux = fwd.tile([N2, XNb], F32, tag="ux")
            vx = fwd.tile([N2, XNb], F32, tag="vx")
            uxv = ux.rearrange("p (c n) -> p c n", c=C)
            vxv = vx.rearrange("p (c n) -> p c n", c=C)

            def bcx(w):
                return w[:, None, :].broadcast_to([N2, C, N1])

            for b in (1, 0):
                nc.vector.tensor_tensor(out=uxv, in0=Axr[:, b], in1=bcx(Twr), op=AL.mult)
                nc.vector.tensor_tensor(out=vxv, in0=Axi[:, b], in1=bcx(Twin), op=AL.mult)
                nc.vector.tensor_tensor(out=Bcv[:, b, :, 0, :], in0=uxv, in1=vxv, op=AL.add)
                nc.vector.tensor_tensor(out=uxv, in0=Axr[:, b], in1=bcx(Twi), op=AL.mult)
                nc.vector.tensor_tensor(out=vxv, in0=Axi[:, b], in1=bcx(Twr), op=AL.mult)
                nc.vector.tensor_tensor(out=Bcv[:, b, :, 1, :], in0=uxv, in1=vxv, op=AL.add)

            # ---------- T1 block transposes (DMA xbar) ----------
            BTH = fwd.tile([N1s, KF], BF16, tag="BTH")             # [(comp n1)|(f,k2)]
            nc.sync.dma_start_transpose(
                out=BTH.rearrange("p (f q) -> p f q", f=FIL), in_=Bh)
            BT = fwd.tile([N1s, XN * 2], BF16, tag="BT")           # [(comp n1)|(b,c,k2)]
            BTv = BT.rearrange("p (b c q) -> p b c q", b=BL, c=C)
            nc.sync.dma_start_transpose(out=BTv[:, 1], in_=Bc[:, CK:])
            nc.sync.dma_start_transpose(out=BTv[:, 0], in_=Bc[:, :CK])

            # ---------- M2h (Hps reuses Ah's banks only) ----------
            phes.close()
            pHes = ExitStack()
            pH = pHes.enter_context(tc.tile_pool(name="pH", bufs=1, space="PSUM"))
            Hps = pH.tile([N1s, KF], F32, tag="Hps")
            for c0, c1 in chunks(KF):
                nc.tensor.matmul(Hps[:, c0:c1], cM2, BTH[:, c0:c1], start=True, stop=True)
            Hs = pers.tile([N1s, KF], F32, tag="Hs")
            sq = pers.tile([N1s, KF], F32, tag="sq")
            HiB = pers.tile([N1, KF], F32, tag="HiB")
            QW = KF // 4
            for q in range(4):
                qs = slice(q * QW, (q + 1) * QW)
                nc.scalar.square(sq[:, qs], Hps[:, qs])
                nc.scalar.copy(out=sqB[:, qs], in_=sq[N1:, qs])
                nc.scalar.copy(out=Hs[:, qs], in_=Hps[:, qs])
                nc.scalar.copy(out=HiB[:, qs], in_=Hs[N1:, qs])

            # ---------- M2x ----------
            pHes.close()
            pxes.close()
            pZes = ExitStack()
            pZ = pZes.enter_context(tc.tile_pool(name="pZ", bufs=1, space="PSUM"))
            Zps = pZ.tile([N1s, XN * 2], F32, tag="Zps")           # [k1s | (b,c,k2)]
            for b in (1, 0):
                for c0, c1 in chunks(CK):
                    nc.tensor.matmul(Zps[:, b * CK + c0:b * CK + c1], cM2,
                                     BT[:, b * CK + c0:b * CK + c1], start=True, stop=True)
                nc.scalar.copy(out=Z0A[:, b * CK:(b + 1) * CK],
                               in_=Zps[:, b * CK:(b + 1) * CK])

            # ---------- G, pipelined per f-quarter (DVE); d in-place in sqB ----------
            def g_quarter(q):
                qs = slice(q * QW, (q + 1) * QW)
                nfq = FIL // 4
                sbv = sqB[:, qs].rearrange("p (f q) -> p f q", f=nfq)
                s64q = s64[:, q * nfq:(q + 1) * nfq, None].broadcast_to(
                    [N1, nfq, N2])
                nc.vector.tensor_tensor(out=sqB[:, qs], in0=sq[:N1, qs],
                                        in1=sqB[:, qs], op=AL.add)
                nc.vector.tensor_tensor(out=sbv, in0=sbv, in1=s64q, op=AL.add)
                nc.vector.reciprocal(out=sq[:N1, qs], in_=sqB[:, qs])
                nc.vector.tensor_tensor(out=G1[:N1, qs], in0=Hs[:N1, qs],
                                        in1=sq[:N1, qs], op=AL.mult)
                nc.vector.tensor_tensor(out=G2[:N1, qs], in0=HiB[:, qs],
                                        in1=sq[:N1, qs], op=AL.mult)
                nc.vector.tensor_copy(out=G1[N1:, qs], in_=G1[:N1, qs])
                nc.vector.tensor_copy(out=G2[N1:, qs], in_=G2[:N1, qs])

            g_quarter(0)

            pZes.close()
            fes.close()

            # ================= inverse =================
            with tc.tile_pool(name="dt", bufs=1) as dtp, \
                 tc.tile_pool(name="stg", bufs=1) as stp, \
                 tc.tile_pool(name="zt", bufs=3) as ztp, \
                 tc.tile_pool(name="cse", bufs=3) as csp, \
                 tc.tile_pool(name="ddp", bufs=2, space="PSUM") as ddp, \
                 tc.tile_pool(name="yp", bufs=4, space="PSUM") as yps:
                DT0 = dtp.tile([N2, N1s * FIL * C], BF16, tag="DT0")
                DT1 = dtp.tile([N2, N1s * FIL * C], BF16, tag="DT1")
                DT = [DT0, DT1]                                    # [k2 | (f,c,n1s')]
                STG0 = stp.tile([N2, N1 * FC], BF16, tag="STG0")
                STG1 = stp.tile([N2, N1 * FC], BF16, tag="STG1")
                STG = [STG0, STG1]                                 # [n2' | (n1',fc)]
                cLv = cL.rearrange("p (m n q) -> p m n q", m=2, n=N1)
                zvA = Z0A.rearrange("p (b c q) -> p b c q", b=BL, c=C)

                def cmul_m3_t2(b, f):
                    eng = nc.gpsimd if (b, f) in POOL_PAIRS else nc.vector
                    g1 = G1[:, f * N2:(f + 1) * N2][:, None, :].broadcast_to([N1s, C, N2])
                    g2 = G2[:, f * N2:(f + 1) * N2][:, None, :].broadcast_to([N1s, C, N2])
                    zt1 = ztp.tile([N1s, CK], BF16, tag="zt1")
                    zt2 = ztp.tile([N1s, CK], BF16, tag="zt2")
                    eng.tensor_tensor(out=zt1.rearrange("p (c q) -> p c q", c=C),
                                      in0=zvA[:, b], in1=g1, op=AL.mult)
                    eng.tensor_tensor(out=zt2.rearrange("p (c q) -> p c q", c=C),
                                      in0=zvA[:, b], in1=g2, op=AL.mult)
                    DD = ddp.tile([N1s, CK], F32, tag="DD")
                    for c0, c1 in chunks(CK):
                        nc.tensor.matmul(DD[:, c0:c1], cM3, zt1[:, c0:c1],
                                         start=True, stop=False)
                        nc.tensor.matmul(DD[:, c0:c1], cM3sw, zt2[:, c0:c1],
                                         start=False, stop=True)
                    cse = csp.tile([N1s, CK], BF16, tag="cse")
                    if EVAC1_DVE(b, f):
                        nc.vector.tensor_copy(out=cse, in_=DD)
                    else:
                        nc.scalar.copy(out=cse, in_=DD)
                    dtv = DT[b].rearrange("p (fi c n) -> p fi c n",
                                          n=N1s, fi=FIL, c=C)[:, f]
                    nc.sync.dma_start_transpose(out=dtv, in_=cse)

                def m4_mm(b, g0, gn=4, seeded=True):
                    dtm = DT[b].rearrange("p (fi c n) -> p n fi c", n=N1s, fi=FIL, c=C)
                    ypsum = yps.tile([N2, gn * FC], F32, tag="yps")
                    for j in range(gn):
                        n1p = g0 + j
                        sl = ypsum[:, j * FC:(j + 1) * FC]
                        if seeded:
                            nc.tensor.matmul(sl, onesr, brow, start=True, stop=False)
                        nc.tensor.matmul(sl, cLv[:, 0, n1p, :], dtm[:, n1p],
                                         start=not seeded, stop=False)
                        nc.tensor.matmul(sl, cLv[:, 1, n1p, :], dtm[:, N1 + n1p],
                                         start=False, stop=True)
                    return ypsum

                def m4_evac(b, g0, ypsum, gn=4, eng="act"):
                    dst = STG[b][:, g0 * FC:(g0 + gn) * FC]
                    if eng == "act":
                        nc.scalar.copy(out=dst, in_=ypsum)
                    else:
                        bb = brep[:, None, :].broadcast_to([N2, gn, FC])
                        nc.vector.tensor_tensor(
                            out=dst.rearrange("p (j fc) -> p j fc", j=gn),
                            in0=ypsum.rearrange("p (j fc) -> p j fc", j=gn),
                            in1=bb, op=AL.add)

                def m4_group(b, g0, gn=4, eng="act"):
                    seeded = eng == "act"
                    m4_evac(b, g0, m4_mm(b, g0, gn, seeded), gn, eng)

                def out_chunk(b, g0, gn=16):
                    nc.scalar.dma_start(
                        out=out_d.ap()[b].rearrange(
                            "(q n) fc -> q (n fc)", n=N1)[:, g0 * FC:(g0 + gn) * FC],
                        in_=STG[b][:, g0 * FC:(g0 + gn) * FC])

                for f in range(FIL):
                    if f in (1, 4, 7):
                        g_quarter(f // 3 + 1)
                    cmul_m3_t2(1, f)
                pend = []
                done1 = 0
                for f in range(FIL):
                    cmul_m3_t2(0, f)
                    # retire deferred b1 evacs (emitted 2 pairs after their MMs)
                    while pend and pend[0][0] <= f - 2:
                        _, g0, yp_t, eng = pend.pop(0)
                        m4_evac(1, g0, yp_t, 4, eng)
                        done1 += 1
                        if done1 == 8:
                            out_chunk(1, 0)
                        elif done1 == 12:
                            out_chunk(1, 16)
                    if f >= 8:
                        for g in range(3):
                            gi = (f - 8) * 3 + g
                            if gi < 16:
                                eng = "dve" if gi % 2 == 0 else "act"
                                yp_t = m4_mm(1, gi * 4, 4, seeded=(eng == "act"))
                                pend.append((f, gi * 4, yp_t, eng))
                for _, g0, yp_t, eng in pend:
                    m4_evac(1, g0, yp_t, 4, eng)
                out_chunk(1, 32)
                out_chunk(1, 48)
                for g0 in range(0, N1, 4):
                    m4_group(0, g0)
                    if g0 % 16 == 12:
                        out_chunk(0, g0 - 12)

    nc.compile()
    return nc


def host_inputs(cfg, x_sh, w_real, w_imag, s, b):
    """Build the per-core in_map (numpy) for one core's batch shard."""
    import ml_dtypes
    cs = host_consts(cfg)
    N1, N2, FIL, C, FC, BL = cfg.N1, cfg.N2, cfg.FIL, cfg.C, cfg.FC, cfg.BL
    f32 = np.float32
    x_sh = np.asarray(x_sh, f32)
    xs = np.ascontiguousarray(
        x_sh.reshape(BL, N2, N1, C).transpose(1, 0, 2, 3)).reshape(N2, BL * N1 * C)
    wr = np.ascontiguousarray(
        np.asarray(w_real, f32).reshape(FIL, N2, N1).transpose(1, 0, 2)).reshape(N2, FIL * N1)
    wi = np.ascontiguousarray(
        np.asarray(w_imag, f32).reshape(FIL, N2, N1).transpose(1, 0, 2)).reshape(N2, FIL * N1)
    blob_f = cs["blob_f"].copy()
    blob_f[:, 3 * N1:] = np.broadcast_to(np.asarray(b, f32).reshape(1, FC), (N2, FC))
    return {
        "xs": xs, "wr": wr, "wi": wi,
        "s64": np.broadcast_to(np.asarray(s, f32).reshape(1, FIL), (N1, FIL)).copy(),
        "brow": np.asarray(b, f32).reshape(1, FC).astype(ml_dtypes.bfloat16),
        "onesr": np.ones((1, N2), f32).astype(ml_dtypes.bfloat16),
        "blob_r": cs["blob_r"],
        "blob_f": blob_f,
        "blob_b": cs["blob_b"].astype(ml_dtypes.bfloat16),
        "c_L": cs["c_L"].astype(ml_dtypes.bfloat16),
    }


_NC_CACHE = {}


def kernel(x, w_real, w_imag, s, b):
    """Full-input entry point: shard over 8 cores, run, gather."""
    from concourse.bass_utils import run_bass_kernel_spmd
    cfg = FULL
    n_cores = 8
    if "full" not in _NC_CACHE:
        _NC_CACHE["full"] = build_nc(cfg)
    nc = _NC_CACHE["full"]
    x = np.asarray(x, dtype=np.float32)
    in_maps = [host_inputs(cfg, x[i * cfg.BL:(i + 1) * cfg.BL], w_real, w_imag, s, b)
               for i in range(n_cores)]
    res = run_bass_kernel_spmd(nc, in_maps, core_ids=list(range(n_cores)))
    outs = [np.asarray(res.results[i]["out"]).astype(np.float32) for i in range(n_cores)]
    return np.concatenate(outs, axis=0)
